# revision 23
# baseline (speedup 1.0000x reference)
"""Trainium2 Bass kernel for nn_AttentionBlock (B=4, S=8192, d=256), 8 cores.

Sharding: data-parallel over batch x sequence-halves. Core c handles batch
b = c // 2 and Q rows [h*4096, (h+1)*4096) with h = c % 2. Each core computes
K/V for its full batch locally (the projection is cheap relative to
attention), so no collectives are needed.

Host-side prep (inside kernel()):
  - x is pre-transposed per batch to xT [d, S] so all projections can contract
    over the partition dim without on-chip transposes.
  - Weights are passed as W.T [d_in, d_out].
  - The V bias commutes through softmax (weights sum to 1), so it is folded
    into an effective output bias bo' = Wo @ bv + bo computed on the host.

On-chip math (per core):
  qT = Wq @ xT + bq   [d, 4096]   (fp32r matmuls, stored bf16)
  kT = Wk @ xT + bk   [d, 8192]   (stored bf16)
  V  = (xT)^T Wv^T    [8192, 257] (stored bf16, col 256 = 1.0 for the denom)
  sT[j, i] = kT[:, j] . qT[:, i]                  (bf16 matmul, psum fp32)
  wT = exp(sT / 16)                               (ScalarE, psum -> sbuf bf16)
  acc[i, :] = sum_j wT[j, i] * V[j, :]            (bf16 matmul, accum in psum)
  y[i, :] = (acc[i, 0:256] @ Wo^T) / acc[i, 256] + bo'   (fp32 tail)

No max-subtraction in the softmax: for these inputs scores are ~N(0, 0.33^2)
(|s/16| < ~3), far inside fp32 exp range; exp/sum in fp32 is exact enough.
"""

import os
from contextlib import ExitStack

import numpy as np

import concourse.bass as bass
import concourse.bacc as bacc
import concourse.mybir as mybir
import concourse.tile as tile
from concourse.masks import make_identity

F32 = mybir.dt.float32
F32R = mybir.dt.float32r
BF16 = mybir.dt.bfloat16
AF = mybir.ActivationFunctionType
ALU = mybir.AluOpType

B = 4
S = 8192
D = 256
NCORES = 8
CORES_PER_BATCH = NCORES // B  # 2
SQ = S // CORES_PER_BATCH      # 4096 Q rows per core

SCALE = float(D) ** -0.5       # 0.0625


def build_nc(s_kv: int = S, s_q: int = SQ, mm_dt=BF16):
    """Build the per-core Bass program. s_kv/s_q parametric for small sims."""
    assert s_kv % 512 == 0 and s_q % 512 == 0
    n_kv_groups = s_kv // 512     # K/V projection groups
    n_q_groups = s_q // 512       # Q projection groups
    n_spans = s_q // 512          # attention i-spans
    n_jb = s_kv // 256            # j-batches (2 j-tiles of 128 each)
    n_jt = s_kv // 128            # total j-tiles

    nc = bacc.Bacc("TRN2", target_bir_lowering=False, debug=False)

    xt_kv = nc.declare_dram_parameter("xt_kv", [D, s_kv], F32R, isOutput=False)
    xt_q = nc.declare_dram_parameter("xt_q", [D, s_q], F32R, isOutput=False)
    wqt = nc.declare_dram_parameter("wqt", [D, D], F32R, isOutput=False)
    wkt = nc.declare_dram_parameter("wkt", [D, D], F32R, isOutput=False)
    wvt = nc.declare_dram_parameter("wvt", [D, D], F32R, isOutput=False)
    wot = nc.declare_dram_parameter("wot", [D, D], F32R, isOutput=False)
    bq2 = nc.declare_dram_parameter("bq2", [128, 2], F32, isOutput=False)
    bk2 = nc.declare_dram_parameter("bk2", [128, 2], F32, isOutput=False)
    bo_bc = nc.declare_dram_parameter("bo_bc", [128, D], F32, isOutput=False)
    y = nc.declare_dram_parameter("y", [s_q, D], F32, isOutput=True)

    with TileKernel(nc) as tk:
        tk.emit(
            xt_kv, xt_q, wqt, wkt, wvt, wot, bq2, bk2, bo_bc, y,
            n_kv_groups, n_q_groups, n_spans, n_jb, n_jt, mm_dt,
        )
    nc.compile()
    return nc


class TileKernel:
    def __init__(self, nc):
        self.nc = nc
        self.ctx = ExitStack()

    def __enter__(self):
        self.tc = self.ctx.enter_context(tile.TileContext(self.nc))
        return self

    def __exit__(self, *exc):
        return self.ctx.__exit__(*exc)

    def emit(self, xt_kv, xt_q, wqt, wkt, wvt, wot, bq2, bk2, bo_bc, y,
             n_kv_groups, n_q_groups, n_spans, n_jb, n_jt, mm_dt):
        nc = self.nc
        tc = self.tc
        ctx = self.ctx
        s_kv = n_jt * 128
        s_q = n_spans * 512

        n_ist = s_q // 128  # total i-subtiles (output row tiles of 128)

        const = ctx.enter_context(tc.tile_pool(name="const", bufs=1))
        persist = ctx.enter_context(tc.tile_pool(name="persist", bufs=1))
        xin = ctx.enter_context(tc.tile_pool(name="xin", bufs=6))
        wexp = ctx.enter_context(tc.tile_pool(name="wexp", bufs=3))
        fin = ctx.enter_context(tc.tile_pool(name="fin", bufs=4))
        yout = ctx.enter_context(tc.tile_pool(name="yout", bufs=3))
        ps_pool = ctx.enter_context(tc.tile_pool(name="ps", bufs=2, space="PSUM"))
        pa_pool = ctx.enter_context(tc.tile_pool(name="pa", bufs=4, space="PSUM"))

        # ---- constants -------------------------------------------------
        # Emission order is tuned for time-to-first-matmul: the first Q
        # x-tile and Wq go out first (the first projection matmul needs
        # exactly those), everything else after.
        xg0 = xin.tile([128, 2, 512], F32R, tag="xg", name="xq_first")
        for c in range(2):
            nc.sync.dma_start(
                out=xg0[:, c, :], in_=xt_q[c * 128:(c + 1) * 128, 0:512]
            )

        def load_weight(name, hbm):
            t = const.tile([128, 2, D], F32R, tag=name, name=name)
            nc.sync.dma_start(out=t, in_=hbm[:].rearrange("(c p) o -> p c o", p=128))
            return t

        wq_sb = load_weight("wq_sb", wqt)
        wk_sb = load_weight("wk_sb", wkt)
        wv_sb = load_weight("wv_sb", wvt)
        wo_sb = load_weight("wo_sb", wot)
        bq_sb = const.tile([128, 2], F32, tag="bq_sb", name="bq_sb")
        nc.sync.dma_start(out=bq_sb, in_=bq2[:])
        bk_sb = const.tile([128, 2], F32, tag="bk_sb", name="bk_sb")
        nc.sync.dma_start(out=bk_sb, in_=bk2[:])
        bo_sb = const.tile([128, D], F32, tag="bo_sb", name="bo_sb")
        nc.sync.dma_start(out=bo_sb, in_=bo_bc[:])
        ident = const.tile([128, 128], F32, tag="ident", name="ident")
        make_identity(nc, ident)

        # ---- persistent activations -----------------------------------
        q_sb = persist.tile([128, 2, s_q], mm_dt, tag="q_sb", name="q_sb")
        k_sb = persist.tile([128, 2, s_kv], mm_dt, tag="k_sb", name="k_sb")
        v_sb = persist.tile([128, n_jt, 257], mm_dt, tag="v_sb", name="v_sb")
        # raw (unnormalized) attention rows + reciprocal denominators,
        # drained from PSUM at each span boundary; consumed by stage 3
        a_st = persist.tile([128, n_ist, 256], F32, tag="a_st", name="a_st")
        rc_st = persist.tile([128, n_ist], F32, tag="rc_st", name="rc_st")
        # ones column for the softmax denominator
        nc.vector.memset(v_sb[:, :, 256:257], 1.0)

        # power-bubble scratch: a chained DVE copy sequence ending in a
        # dummy ldweights gives the PE a deterministic ~2.8us idle window
        # at each span boundary. Without these, the sustained-power
        # throttler drops the PE clock 2.4 -> 2.0 GHz for the whole run
        # (measured: 606us with idle gaps vs 700us fully dense). Kept
        # under ~3.4us so the HAM activity monitor never re-throttles.
        bub_a = persist.tile([128, 256], F32, tag="bub_a", name="bub_a")
        bub_b = persist.tile([128, 256], F32, tag="bub_b", name="bub_b")
        bub_w = persist.tile([128, 128], mm_dt, tag="bub_w", name="bub_w")

        def bubble(src_ap, n_hops=7):
            nc.vector.tensor_copy(out=bub_a, in_=src_ap)
            cur, nxt = bub_a, bub_b
            for _ in range(n_hops):
                nc.vector.tensor_copy(out=nxt, in_=cur)
                cur, nxt = nxt, cur
            nc.vector.tensor_copy(out=bub_w, in_=cur[:, 0:128])
            nc.tensor.ldweights(bub_w)

        def r32(ap):
            return ap.bitcast(F32R)

        # ---- Q projection ---------------------------------------------
        for g in range(n_q_groups):
            if g == 0:
                xg = xg0
            else:
                xg = xin.tile([128, 2, 512], F32R, tag="xg", name=f"xq{g}")
                for c in range(2):
                    nc.sync.dma_start(
                        out=xg[:, c, :],
                        in_=xt_q[c * 128:(c + 1) * 128, g * 512:(g + 1) * 512],
                    )
            for c2 in range(2):
                pk = ps_pool.tile([128, 512], F32, tag="ps", name=f"pq{g}_{c2}")
                for c in range(2):
                    nc.tensor.matmul(
                        pk,
                        lhsT=wq_sb[:, c, c2 * 128:(c2 + 1) * 128],
                        rhs=xg[:, c, :],
                        start=(c == 0), stop=(c == 1),
                    )
                nc.scalar.activation(
                    out=q_sb[:, c2, g * 512:(g + 1) * 512], in_=pk,
                    func=AF.Identity, bias=bq_sb[:, c2:c2 + 1],
                )

        # ---- K / V projections ----------------------------------------
        for g in range(n_kv_groups):
            xg = xin.tile([128, 2, 512], F32R, tag="xg", name=f"xkv{g}")
            for c in range(2):
                nc.sync.dma_start(
                    out=xg[:, c, :],
                    in_=xt_kv[c * 128:(c + 1) * 128, g * 512:(g + 1) * 512],
                )
            for c2 in range(2):
                pk = ps_pool.tile([128, 512], F32, tag="ps", name=f"pk{g}_{c2}")
                for c in range(2):
                    nc.tensor.matmul(
                        pk,
                        lhsT=wk_sb[:, c, c2 * 128:(c2 + 1) * 128],
                        rhs=xg[:, c, :],
                        start=(c == 0), stop=(c == 1),
                    )
                nc.scalar.activation(
                    out=k_sb[:, c2, g * 512:(g + 1) * 512], in_=pk,
                    func=AF.Identity, bias=bk_sb[:, c2:c2 + 1],
                )
            for t in range(4):
                pv = pa_pool.tile([128, 256], F32, tag="pa", name=f"pv{g}_{t}")
                for c in range(2):
                    nc.tensor.matmul(
                        pv,
                        lhsT=xg[:, c, t * 128:(t + 1) * 128],
                        rhs=wv_sb[:, c, :],
                        start=(c == 0), stop=(c == 1),
                    )
                nc.vector.tensor_copy(out=v_sb[:, g * 4 + t, 0:256], in_=pv)

        # ---- attention -------------------------------------------------
        def finalize(sp):
            """Normalize + output-project span sp's accumulators.

            Emitted after the NEXT span's first two scores batches so the
            PE transition across the span boundary stays busy. PSUM for the
            transposes / O-proj comes from the pa pool — the slots this
            very finalize releases — so the scores pipeline is never
            starved of ps slots. The DVE copies run t-ascending, releasing
            pa slots one at a time.
            """
            pa_t = pa_tiles[sp]
            for t in range(4):
                rec = fin.tile([128, 1], F32, tag="rec", name=f"rec{sp}_{t}")
                nc.vector.reciprocal(out=rec, in_=pa_t[t][:, 256:257])
                a_sb = fin.tile([128, 256], F32, tag="a", name=f"a{sp}_{t}")
                nc.vector.tensor_copy(out=a_sb, in_=pa_t[t][:, 0:256])
                at_sb = fin.tile([128, 2, 128], F32R, tag="at", name=f"at{sp}_{t}")
                for c in range(2):
                    tp = pa_pool.tile([128, 128], F32, tag="pa", name=f"tp{sp}_{t}_{c}")
                    nc.tensor.transpose(tp, a_sb[:, c * 128:(c + 1) * 128], ident)
                    if c == 0:
                        nc.vector.tensor_copy(out=at_sb[:, c, :], in_=tp)
                    else:
                        nc.scalar.copy(out=at_sb[:, c, :], in_=tp)
                py = pa_pool.tile([128, 256], F32, tag="pa", name=f"py{sp}_{t}")
                for c in range(2):
                    nc.tensor.matmul(
                        py,
                        lhsT=at_sb[:, c, :],
                        rhs=wo_sb[:, c, :],
                        start=(c == 0), stop=(c == 1),
                    )
                y_sb = yout.tile([128, 256], F32, tag="y", name=f"y{sp}_{t}")
                nc.vector.scalar_tensor_tensor(
                    out=y_sb, in0=py, scalar=rec, in1=bo_sb,
                    op0=ALU.mult, op1=ALU.add,
                )
                i0 = (sp * 4 + t) * 128
                nc.sync.dma_start(out=y[i0:i0 + 128, :], in_=y_sb)

        # power bubble between the projections and the attention spans
        bubble(k_sb[:, 1, s_kv - 256:s_kv])

        pa_tiles = {}
        for sp in range(n_spans):
            qs = q_sb[:, :, sp * 512:(sp + 1) * 512]

            def scores(jb):
                ps = ps_pool.tile([128, 1024], F32, tag="ps", name=f"ps{sp}_{jb}")
                for u in range(2):
                    jt = jb * 2 + u
                    for c2 in range(2):
                        nc.tensor.matmul(
                            ps[:, u * 512:(u + 1) * 512],
                            lhsT=k_sb[:, c2, jt * 128:(jt + 1) * 128],
                            rhs=qs[:, c2, :],
                            start=(c2 == 0), stop=(c2 == 1),
                        )
                w = wexp.tile([128, 1024], mm_dt, tag="w", name=f"w{sp}_{jb}")
                nc.scalar.activation(out=w, in_=ps, func=AF.Exp, scale=SCALE)
                return w

            def attn(jb, w):
                for u in range(2):
                    jt = jb * 2 + u
                    for t in range(4):
                        nc.tensor.matmul(
                            pa_t[t],
                            lhsT=w[:, u * 512 + t * 128:u * 512 + (t + 1) * 128],
                            rhs=v_sb[:, jt, :],
                            start=(jb == 0 and u == 0),
                            stop=(jb == n_jb - 1 and u == 1),
                        )

            # software pipeline: scores(0), scores(1) first, then the
            # PREVIOUS span's finalize (its PE work overlaps these scores'
            # exp latency), then this span's accumulators, then the j-loop
            # with scores(jb+1) emitted ahead of attn(jb). The full-idle
            # power bubble at the span end keeps the sustained-power
            # throttler from dropping the PE clock to 2.0 GHz (a finalize
            # window alone is not enough relief - measured).
            w0 = scores(0)
            w1 = scores(1)
            if sp > 0:
                finalize(sp - 1)
            pa_t = [
                pa_pool.tile([128, 257], F32, tag="pa", name=f"pa{sp}_{t}")
                for t in range(4)
            ]
            pa_tiles[sp] = pa_t
            attn(0, w0)
            w_prev = w1
            for jb in range(2, n_jb):
                w_cur = scores(jb)
                attn(jb - 1, w_prev)
                w_prev = w_cur
            attn(n_jb - 1, w_prev)
            if sp < n_spans - 1:
                bubble(pa_t[3][:, 0:256])

        finalize(n_spans - 1)


def _host_prep(x, Wq, bq, Wk, bk, Wv, bv, Wo, bo):
    """Shared host-side preprocessing -> list of per-core input maps."""
    f = lambda a: np.asarray(a, dtype=np.float32)
    x = f(x)
    xt = np.ascontiguousarray(np.transpose(x, (0, 2, 1)))  # [B, D, S]
    wqt = np.ascontiguousarray(f(Wq).T)
    wkt = np.ascontiguousarray(f(Wk).T)
    wvt = np.ascontiguousarray(f(Wv).T)
    wot = np.ascontiguousarray(f(Wo).T)
    bq2 = np.ascontiguousarray(f(bq).reshape(2, 128).T)
    bk2 = np.ascontiguousarray(f(bk).reshape(2, 128).T)
    bo_eff = f(Wo) @ f(bv) + f(bo)
    bo_bc = np.ascontiguousarray(np.tile(bo_eff[None, :], (128, 1)))

    in_maps = []
    for c in range(NCORES):
        b, h = divmod(c, CORES_PER_BATCH)
        in_maps.append({
            "xt_kv": xt[b],
            "xt_q": np.ascontiguousarray(xt[b][:, h * SQ:(h + 1) * SQ]),
            "wqt": wqt, "wkt": wkt, "wvt": wvt, "wot": wot,
            "bq2": bq2, "bk2": bk2, "bo_bc": bo_bc,
        })
    return in_maps


_NC_CACHE = {}


def _get_nc():
    if "nc" not in _NC_CACHE:
        _NC_CACHE["nc"] = build_nc()
    return _NC_CACHE["nc"]


def kernel(x, Wq, bq, Wk, bk, Wv, bv, Wo, bo):
    from concourse.bass_utils import run_bass_kernel_spmd

    nc = _get_nc()
    in_maps = _host_prep(x, Wq, bq, Wk, bk, Wv, bv, Wo, bo)
    res = run_bass_kernel_spmd(nc, in_maps, list(range(NCORES)))
    out = np.empty((B, S, D), dtype=np.float32)
    for c in range(NCORES):
        b, h = divmod(c, CORES_PER_BATCH)
        out[b, h * SQ:(h + 1) * SQ, :] = res.results[c]["y"]
    return out


# revision 25
# speedup vs baseline: 1.1957x; 1.1957x over previous
"""Trainium2 Bass kernel for nn_AttentionBlock (B=4, S=8192, d=256), 8 cores.

Sharding: data-parallel over batch x sequence-halves. Core c handles batch
b = c // 2 and Q rows [h*4096, (h+1)*4096) with h = c % 2. Each core computes
K/V for its full batch locally (the projection is cheap relative to
attention), so no collectives are needed.

Host-side prep (inside kernel()):
  - x is pre-transposed per batch to xT [d, S] so all projections can contract
    over the partition dim without on-chip transposes.
  - Weights are passed as W.T [d_in, d_out].
  - The V bias commutes through softmax (weights sum to 1), so it is folded
    into an effective output bias bo' = Wo @ bv + bo computed on the host.

On-chip math (per core):
  qT = Wq @ xT + bq   [d, 4096]   (fp32r matmuls, stored bf16)
  kT = Wk @ xT + bk   [d, 8192]   (stored bf16)
  V  = (xT)^T Wv^T    [8192, 257] (stored bf16, col 256 = 1.0 for the denom)
  sT[j, i] = kT[:, j] . qT[:, i]                  (bf16 matmul, psum fp32)
  wT = exp(sT / 16)                               (ScalarE, psum -> sbuf bf16)
  acc[i, :] = sum_j wT[j, i] * V[j, :]            (bf16 matmul, accum in psum)
  y[i, :] = (acc[i, 0:256] @ Wo^T) / acc[i, 256] + bo'   (fp32 tail)

No max-subtraction in the softmax: for these inputs scores are ~N(0, 0.33^2)
(|s/16| < ~3), far inside fp32 exp range; exp/sum in fp32 is exact enough.
"""

from contextlib import ExitStack

import numpy as np

import concourse.bass as bass
import concourse.bacc as bacc
import concourse.mybir as mybir
import concourse.tile as tile
from concourse.masks import make_identity

F32 = mybir.dt.float32
F32R = mybir.dt.float32r
BF16 = mybir.dt.bfloat16
AF = mybir.ActivationFunctionType
ALU = mybir.AluOpType

B = 4
S = 8192
D = 256
NCORES = 8
CORES_PER_BATCH = NCORES // B  # 2
SQ = S // CORES_PER_BATCH      # 4096 Q rows per core

SCALE = float(D) ** -0.5       # 0.0625


def build_nc(s_kv: int = S, s_q: int = SQ, mm_dt=BF16):
    """Build the per-core Bass program. s_kv/s_q parametric for small sims."""
    assert s_kv % 512 == 0 and s_q % 512 == 0
    n_kv_groups = s_kv // 512     # K/V projection groups
    n_q_groups = s_q // 512       # Q projection groups
    n_spans = s_q // 512          # attention i-spans
    n_jb = s_kv // 256            # j-batches (2 j-tiles of 128 each)
    n_jt = s_kv // 128            # total j-tiles

    nc = bacc.Bacc("TRN2", target_bir_lowering=False, debug=False)

    xt_kv = nc.declare_dram_parameter("xt_kv", [D, s_kv], F32R, isOutput=False)
    xt_q = nc.declare_dram_parameter("xt_q", [D, s_q], F32R, isOutput=False)
    wqt = nc.declare_dram_parameter("wqt", [D, D], F32R, isOutput=False)
    wkt = nc.declare_dram_parameter("wkt", [D, D], F32R, isOutput=False)
    wvt = nc.declare_dram_parameter("wvt", [D, D], F32R, isOutput=False)
    wot = nc.declare_dram_parameter("wot", [D, D], F32R, isOutput=False)
    bq2 = nc.declare_dram_parameter("bq2", [128, 2], F32, isOutput=False)
    bk2 = nc.declare_dram_parameter("bk2", [128, 2], F32, isOutput=False)
    bo_bc = nc.declare_dram_parameter("bo_bc", [128, D], F32, isOutput=False)
    y = nc.declare_dram_parameter("y", [s_q, D], F32, isOutput=True)

    with TileKernel(nc) as tk:
        tk.emit(
            xt_kv, xt_q, wqt, wkt, wvt, wot, bq2, bk2, bo_bc, y,
            n_kv_groups, n_q_groups, n_spans, n_jb, n_jt, mm_dt,
        )
    nc.compile()
    return nc


class TileKernel:
    def __init__(self, nc):
        self.nc = nc
        self.ctx = ExitStack()

    def __enter__(self):
        self.tc = self.ctx.enter_context(tile.TileContext(self.nc))
        return self

    def __exit__(self, *exc):
        return self.ctx.__exit__(*exc)

    def emit(self, xt_kv, xt_q, wqt, wkt, wvt, wot, bq2, bk2, bo_bc, y,
             n_kv_groups, n_q_groups, n_spans, n_jb, n_jt, mm_dt):
        nc = self.nc
        tc = self.tc
        ctx = self.ctx
        s_kv = n_jt * 128
        s_q = n_spans * 512

        const = ctx.enter_context(tc.tile_pool(name="const", bufs=1))
        persist = ctx.enter_context(tc.tile_pool(name="persist", bufs=1))
        xin = ctx.enter_context(tc.tile_pool(name="xin", bufs=6))
        wexp = ctx.enter_context(tc.tile_pool(name="wexp", bufs=3))
        fin = ctx.enter_context(tc.tile_pool(name="fin", bufs=4))
        yout = ctx.enter_context(tc.tile_pool(name="yout", bufs=3))
        ps_pool = ctx.enter_context(tc.tile_pool(name="ps", bufs=2, space="PSUM"))
        pa_pool = ctx.enter_context(tc.tile_pool(name="pa", bufs=4, space="PSUM"))

        # ---- constants -------------------------------------------------
        # Emission order is tuned for time-to-first-matmul: the first Q
        # x-tile and Wq go out first (the first projection matmul needs
        # exactly those), everything else after.
        xg0 = xin.tile([128, 2, 512], F32R, tag="xg", name="xq_first")
        for c in range(2):
            nc.sync.dma_start(
                out=xg0[:, c, :], in_=xt_q[c * 128:(c + 1) * 128, 0:512]
            )

        def load_weight(name, hbm):
            t = const.tile([128, 2, D], F32R, tag=name, name=name)
            nc.sync.dma_start(out=t, in_=hbm[:].rearrange("(c p) o -> p c o", p=128))
            return t

        wq_sb = load_weight("wq_sb", wqt)
        wk_sb = load_weight("wk_sb", wkt)
        wv_sb = load_weight("wv_sb", wvt)
        wo_sb = load_weight("wo_sb", wot)
        bq_sb = const.tile([128, 2], F32, tag="bq_sb", name="bq_sb")
        nc.sync.dma_start(out=bq_sb, in_=bq2[:])
        bk_sb = const.tile([128, 2], F32, tag="bk_sb", name="bk_sb")
        nc.sync.dma_start(out=bk_sb, in_=bk2[:])
        bo_sb = const.tile([128, D], F32, tag="bo_sb", name="bo_sb")
        nc.sync.dma_start(out=bo_sb, in_=bo_bc[:])
        ident = const.tile([128, 128], F32, tag="ident", name="ident")
        make_identity(nc, ident)

        # ---- persistent activations -----------------------------------
        q_sb = persist.tile([128, 2, s_q], mm_dt, tag="q_sb", name="q_sb")
        k_sb = persist.tile([128, 2, s_kv], mm_dt, tag="k_sb", name="k_sb")
        v_sb = persist.tile([128, n_jt, 257], mm_dt, tag="v_sb", name="v_sb")
        # ones column for the softmax denominator
        nc.vector.memset(v_sb[:, :, 256:257], 1.0)

        # power-bubble scratch: a chained DVE copy sequence ending in a
        # dummy ldweights gives the PE a deterministic ~2.8us idle window
        # at each span boundary. Without these, the sustained-power
        # throttler drops the PE clock 2.4 -> 2.0 GHz for the whole run
        # (measured: 606us with idle gaps vs 700us fully dense). Kept
        # under ~3.4us so the HAM activity monitor never re-throttles.
        bub_a = persist.tile([128, 256], F32, tag="bub_a", name="bub_a")
        bub_b = persist.tile([128, 256], F32, tag="bub_b", name="bub_b")
        bub_w = persist.tile([128, 128], mm_dt, tag="bub_w", name="bub_w")

        def bubble(src_ap, n_hops=7):
            nc.vector.tensor_copy(out=bub_a, in_=src_ap)
            cur, nxt = bub_a, bub_b
            for _ in range(n_hops):
                nc.vector.tensor_copy(out=nxt, in_=cur)
                cur, nxt = nxt, cur
            nc.vector.tensor_copy(out=bub_w, in_=cur[:, 0:128])
            nc.tensor.ldweights(bub_w)

        # ---- Q projection ---------------------------------------------
        for g in range(n_q_groups):
            if g == 0:
                xg = xg0
            else:
                xg = xin.tile([128, 2, 512], F32R, tag="xg", name=f"xq{g}")
                for c in range(2):
                    nc.sync.dma_start(
                        out=xg[:, c, :],
                        in_=xt_q[c * 128:(c + 1) * 128, g * 512:(g + 1) * 512],
                    )
            for c2 in range(2):
                pk = ps_pool.tile([128, 512], F32, tag="ps", name=f"pq{g}_{c2}")
                for c in range(2):
                    nc.tensor.matmul(
                        pk,
                        lhsT=wq_sb[:, c, c2 * 128:(c2 + 1) * 128],
                        rhs=xg[:, c, :],
                        start=(c == 0), stop=(c == 1),
                    )
                nc.scalar.activation(
                    out=q_sb[:, c2, g * 512:(g + 1) * 512], in_=pk,
                    func=AF.Identity, bias=bq_sb[:, c2:c2 + 1],
                )

        # ---- K / V projections ----------------------------------------
        for g in range(n_kv_groups):
            xg = xin.tile([128, 2, 512], F32R, tag="xg", name=f"xkv{g}")
            for c in range(2):
                nc.sync.dma_start(
                    out=xg[:, c, :],
                    in_=xt_kv[c * 128:(c + 1) * 128, g * 512:(g + 1) * 512],
                )
            for c2 in range(2):
                pk = ps_pool.tile([128, 512], F32, tag="ps", name=f"pk{g}_{c2}")
                for c in range(2):
                    nc.tensor.matmul(
                        pk,
                        lhsT=wk_sb[:, c, c2 * 128:(c2 + 1) * 128],
                        rhs=xg[:, c, :],
                        start=(c == 0), stop=(c == 1),
                    )
                nc.scalar.activation(
                    out=k_sb[:, c2, g * 512:(g + 1) * 512], in_=pk,
                    func=AF.Identity, bias=bk_sb[:, c2:c2 + 1],
                )
            for t in range(4):
                pv = pa_pool.tile([128, 256], F32, tag="pa", name=f"pv{g}_{t}")
                for c in range(2):
                    nc.tensor.matmul(
                        pv,
                        lhsT=xg[:, c, t * 128:(t + 1) * 128],
                        rhs=wv_sb[:, c, :],
                        start=(c == 0), stop=(c == 1),
                    )
                nc.vector.tensor_copy(out=v_sb[:, g * 4 + t, 0:256], in_=pv)

        # ---- attention -------------------------------------------------
        def finalize(sp):
            """Normalize + output-project span sp's accumulators.

            Emitted after the NEXT span's first two scores batches so the
            PE transition across the span boundary stays busy. PSUM for the
            transposes / O-proj comes from the pa pool — the slots this
            very finalize releases — so the scores pipeline is never
            starved of ps slots. The DVE copies run t-ascending, releasing
            pa slots one at a time.
            """
            pa_t = pa_tiles[sp]
            for t in range(4):
                rec = fin.tile([128, 1], F32, tag="rec", name=f"rec{sp}_{t}")
                nc.vector.reciprocal(out=rec, in_=pa_t[t][:, 256:257])
                a_sb = fin.tile([128, 256], F32, tag="a", name=f"a{sp}_{t}")
                nc.vector.tensor_copy(out=a_sb, in_=pa_t[t][:, 0:256])
                at_sb = fin.tile([128, 2, 128], F32R, tag="at", name=f"at{sp}_{t}")
                for c in range(2):
                    tp = pa_pool.tile([128, 128], F32, tag="pa", name=f"tp{sp}_{t}_{c}")
                    nc.tensor.transpose(tp, a_sb[:, c * 128:(c + 1) * 128], ident)
                    if c == 0:
                        nc.vector.tensor_copy(out=at_sb[:, c, :], in_=tp)
                    else:
                        nc.scalar.copy(out=at_sb[:, c, :], in_=tp)
                py = pa_pool.tile([128, 256], F32, tag="pa", name=f"py{sp}_{t}")
                for c in range(2):
                    nc.tensor.matmul(
                        py,
                        lhsT=at_sb[:, c, :],
                        rhs=wo_sb[:, c, :],
                        start=(c == 0), stop=(c == 1),
                    )
                y_sb = yout.tile([128, 256], F32, tag="y", name=f"y{sp}_{t}")
                nc.vector.scalar_tensor_tensor(
                    out=y_sb, in0=py, scalar=rec, in1=bo_sb,
                    op0=ALU.mult, op1=ALU.add,
                )
                i0 = (sp * 4 + t) * 128
                nc.sync.dma_start(out=y[i0:i0 + 128, :], in_=y_sb)

        # power bubble between the projections and the attention spans
        bubble(k_sb[:, 1, s_kv - 256:s_kv])

        pa_tiles = {}
        for sp in range(n_spans):
            qs = q_sb[:, :, sp * 512:(sp + 1) * 512]

            def scores(jb):
                ps = ps_pool.tile([128, 1024], F32, tag="ps", name=f"ps{sp}_{jb}")
                for u in range(2):
                    jt = jb * 2 + u
                    for c2 in range(2):
                        nc.tensor.matmul(
                            ps[:, u * 512:(u + 1) * 512],
                            lhsT=k_sb[:, c2, jt * 128:(jt + 1) * 128],
                            rhs=qs[:, c2, :],
                            start=(c2 == 0), stop=(c2 == 1),
                        )
                w = wexp.tile([128, 1024], mm_dt, tag="w", name=f"w{sp}_{jb}")
                nc.scalar.activation(out=w, in_=ps, func=AF.Exp, scale=SCALE)
                return w

            def attn(jb, w):
                for u in range(2):
                    jt = jb * 2 + u
                    for t in range(4):
                        nc.tensor.matmul(
                            pa_t[t],
                            lhsT=w[:, u * 512 + t * 128:u * 512 + (t + 1) * 128],
                            rhs=v_sb[:, jt, :],
                            start=(jb == 0 and u == 0),
                            stop=(jb == n_jb - 1 and u == 1),
                        )

            # software pipeline: scores(0), scores(1) first, then the
            # PREVIOUS span's finalize (its PE work overlaps these scores'
            # exp latency), then this span's accumulators, then the j-loop
            # with scores(jb+1) emitted ahead of attn(jb). The full-idle
            # power bubble at the span end keeps the sustained-power
            # throttler from dropping the PE clock to 2.0 GHz (a finalize
            # window alone is not enough relief - measured).
            w0 = scores(0)
            w1 = scores(1)
            if sp > 0:
                finalize(sp - 1)
            pa_t = [
                pa_pool.tile([128, 257], F32, tag="pa", name=f"pa{sp}_{t}")
                for t in range(4)
            ]
            pa_tiles[sp] = pa_t
            attn(0, w0)
            w_prev = w1
            for jb in range(2, n_jb):
                w_cur = scores(jb)
                attn(jb - 1, w_prev)
                w_prev = w_cur
            attn(n_jb - 1, w_prev)
            if sp < n_spans - 1:
                bubble(pa_t[3][:, 0:256])

        finalize(n_spans - 1)


def _host_prep(x, Wq, bq, Wk, bk, Wv, bv, Wo, bo):
    """Shared host-side preprocessing -> list of per-core input maps."""
    f = lambda a: np.asarray(a, dtype=np.float32)
    x = f(x)
    xt = np.ascontiguousarray(np.transpose(x, (0, 2, 1)))  # [B, D, S]
    wqt = np.ascontiguousarray(f(Wq).T)
    wkt = np.ascontiguousarray(f(Wk).T)
    wvt = np.ascontiguousarray(f(Wv).T)
    wot = np.ascontiguousarray(f(Wo).T)
    bq2 = np.ascontiguousarray(f(bq).reshape(2, 128).T)
    bk2 = np.ascontiguousarray(f(bk).reshape(2, 128).T)
    bo_eff = f(Wo) @ f(bv) + f(bo)
    bo_bc = np.ascontiguousarray(np.tile(bo_eff[None, :], (128, 1)))

    in_maps = []
    for c in range(NCORES):
        b, h = divmod(c, CORES_PER_BATCH)
        in_maps.append({
            "xt_kv": xt[b],
            "xt_q": np.ascontiguousarray(xt[b][:, h * SQ:(h + 1) * SQ]),
            "wqt": wqt, "wkt": wkt, "wvt": wvt, "wot": wot,
            "bq2": bq2, "bk2": bk2, "bo_bc": bo_bc,
        })
    return in_maps


_NC_CACHE = {}


def _get_nc():
    if "nc" not in _NC_CACHE:
        _NC_CACHE["nc"] = build_nc()
    return _NC_CACHE["nc"]


def kernel(x, Wq, bq, Wk, bk, Wv, bv, Wo, bo):
    from concourse.bass_utils import run_bass_kernel_spmd

    nc = _get_nc()
    in_maps = _host_prep(x, Wq, bq, Wk, bk, Wv, bv, Wo, bo)
    res = run_bass_kernel_spmd(nc, in_maps, list(range(NCORES)))
    out = np.empty((B, S, D), dtype=np.float32)
    for c in range(NCORES):
        b, h = divmod(c, CORES_PER_BATCH)
        out[b, h * SQ:(h + 1) * SQ, :] = res.results[c]["y"]
    return out
